# revision 2
# baseline (speedup 1.0000x reference)
"""MultiHeadDenseAttention on 8 Trainium2 NeuronCores — v2 (bf16).

Head-sharded tensor parallelism: each core computes 2 of 16 heads, then an
AllToAll exchanges head-blocks for row-blocks and each core applies the
output projection to its 512 rows.

v2 changes vs baseline:
  - bf16 matmul inputs everywhere (halves DMA); f32 PSUM accumulation.
  - value projection emits vh [m,d] chunks directly (m on out partitions,
    PSUM-accumulated over k) — no PE transposes, no SBUF vacc adds.
  - b2 folded into vh via exp(b2) scaling (the vh write is a
    tensor_scalar_mul); softmax denominator rides as a ones column in vh.
  - relu + normalize on DVE, exp alone on ACT (the 16.8M-element exp pass
    is the second-largest engine load after PE).
  - act normalized pre-A2A: the exchange carries bf16 [64,512] tiles and
    P4 is just recv-DMA + out-proj matmuls.
  - software-pipelined P2: S@V(nb-1) emitted after logits(nb); value
    projection of batch 1 interleaved into head-0 blocks.

Layouts (per core c, heads 2c / 2c+1):
  xt tiles [8][128, 4096]  xT k-chunks (bf16)
  xc   [128, 4096]  xT rows [128c, 128c+128) = this core's heads' x dims
  wv   [128, 1024]  k-chunk f at cols [128f:128f+128) = Wv.T[kchunk, our cols]
  w1blk [128, 128]  block-diag W1.T (both heads)
  w2t  [128, 2048]  W2.T stacked twice (h0 rows 0:64, h1 rows 64:128)
  vh   [128, 2*16*132] bf16: per (batch, m-chunk): [h0 v64,den,den | h1 ...]
  hidT [128, 4096]  relu(W1 xh + b1), both heads on partitions
  po   [66, 512]    S@V accum: rows 0:64 = sum(exp*v), rows 64:66 = sum(exp)
  a2a  [8, 64, 512] bf16 normalized act blocks
"""

import sys

if "/opt/trn_rl_repo" not in sys.path:
    sys.path.insert(0, "/opt/trn_rl_repo")

from contextlib import ExitStack

import ml_dtypes
import numpy as np

import bass_rust
import concourse.bass as bass
import concourse.tile as tile
from concourse import mybir
from concourse.bass_utils import run_bass_kernel_spmd

F32 = mybir.dt.float32
F32R = mybir.dt.float32r
BF16 = mybir.dt.bfloat16
AF = mybir.ActivationFunctionType
ALU = mybir.AluOpType

NC = 8            # cores
B = 2             # batch
N_SEQ = 2048      # seq len == max_seq_len (m)
FEAT = 1024
H = 16            # heads
D = 64            # head dim
NTOT = B * N_SEQ  # 4096 flattened rows
NBLK = 512        # n-block size
NB = NTOT // NBLK # 8 n-blocks (== A2A shards == cores)
MC = N_SEQ // 128 # 16 m-chunks per batch
CB = 132          # vh per-chunk stride: 66 (h0+den,den) + 66 (h1+den,den)


def _split_sem_waits(nc, limit=1):
    """Walrus rejects instructions with more than ~1 sync wait; move the
    excess onto NOPs on the same engine inserted immediately before."""
    blocks = {}
    for f in nc.m.functions:
        for bb in f.blocks:
            blocks[bb.name] = bb
    for bb in blocks.values():
        i = 0
        while i < len(bb.instructions):
            inst = bb.instructions[i]
            si = inst.sync_info
            if si is not None and si.on_wait and len(si.on_wait) > limit:
                waits = list(si.on_wait)
                chunks = [waits[j : j + limit] for j in range(0, len(waits), limit)]
                si.on_wait = chunks[-1]
                engine = nc.engines[inst.engine]
                for chunk in chunks[:-1]:
                    d = engine.nop(nofuse=True, hint="wait_split")
                    dinst = d.ins if hasattr(d, "ins") else d
                    for ob in blocks.values():
                        if ob.instructions and ob.instructions[-1] is dinst:
                            ob.instructions.pop()
                            break
                    dinst.sync_info = bass_rust.SyncInfo(on_wait=chunk, on_update=[])
                    bb.instructions.insert(i, dinst)
                    i += 1
            i += 1
    return nc


def _build(reps=1, phases="A"):
    nc = bass.Bass()

    xt_in = nc.dram_tensor("xt", [FEAT, NTOT], BF16, kind="ExternalInput")
    xc_in = nc.dram_tensor("xc", [128, NTOT], BF16, kind="ExternalInput")
    wv_in = nc.dram_tensor("wv", [128, FEAT], BF16, kind="ExternalInput")
    w1_in = nc.dram_tensor("w1blk", [128, 128], BF16, kind="ExternalInput")
    b1_in = nc.dram_tensor("b1", [128, 1], F32, kind="ExternalInput")
    w2t_in = nc.dram_tensor("w2t", [128, N_SEQ], BF16, kind="ExternalInput")
    eb2_in = nc.dram_tensor("eb2", [128, MC], F32, kind="ExternalInput")
    wot_in = nc.dram_tensor("wot", [128, NC * FEAT], BF16, kind="ExternalInput")
    out_ext = nc.dram_tensor("out", [NBLK, FEAT], BF16, kind="ExternalOutput")

    with tile.TileContext(nc) as tc, ExitStack() as ctx:
        wp = ctx.enter_context(tc.tile_pool(name="wp", bufs=1))
        dram = ctx.enter_context(tc.tile_pool(name="dram", bufs=1, space="DRAM"))

        # ---- resident weights/constants -------------------------------
        wv = wp.tile([128, FEAT], BF16)
        nc.sync.dma_start(wv[:], wv_in[:])
        w1blk = wp.tile([128, 128], BF16)
        nc.sync.dma_start(w1blk[:], w1_in[:])
        b1t = wp.tile([128, 1], F32)
        nc.sync.dma_start(b1t[:], b1_in[:])
        w2t = wp.tile([128, N_SEQ], BF16)
        nc.sync.dma_start(w2t[:], w2t_in[:])
        eb2 = wp.tile([128, MC], F32)
        nc.sync.dma_start(eb2[:], eb2_in[:])
        wot = wp.tile([128, NC * FEAT], BF16)

        # broadcast weights: rows 64:66 both 0.5 — po carries the exp-sum
        # twice, so pb = 0.5/den + 0.5/den = 1/den. f32r via copy (memset
        # on f32r is invalid ISA)
        ones2f = wp.tile([66, 128], F32)
        nc.vector.memset(ones2f[64:66, :], 0.5)
        ones2 = wp.tile([66, 128], F32R)
        nc.vector.tensor_copy(ones2[64:66, :], ones2f[64:66, :])

        # vh: [128, B*MC*CB]; den columns (64, 65+64) = exp(b2), written once
        vh = wp.tile([128, B * MC * CB], BF16, name="vh", tag="vh")

        def vh_ap(b, j, lo, hi):
            base = (b * MC + j) * CB
            return vh[:, base + lo : base + hi]

        for b in range(B):
            for j in range(MC):
                for h in range(2):
                    for dd in range(2):
                        nc.vector.tensor_scalar_mul(
                            vh_ap(b, j, h * 66 + D + dd, h * 66 + D + dd + 1),
                            eb2[:, j : j + 1],
                            1.0,
                        )

        # persistent per-rep tiles
        xtp = ctx.enter_context(tc.tile_pool(name="xtp", bufs=1))
        xts = [xtp.tile([128, NTOT], BF16, name=f"xt{k}", tag=f"xt{k}") for k in range(8)]
        xc = xtp.tile([128, NTOT], BF16, name="xc", tag="xc")
        hidT = xtp.tile([128, NTOT], BF16, name="hidT", tag="hidT")

        for _rep in range(reps):
            a2a_send = [dram.tile([NC, D, NBLK], BF16, name=f"snd{h}_{_rep}") for h in range(2)]
            a2a_recv = [dram.tile([NC, D, NBLK], BF16, name=f"rcv{h}_{_rep}") for h in range(2)]

            cr = ExitStack()
            with cr:
              sp = cr.enter_context(tc.tile_pool(name="sp", bufs=1))
              acts = [sp.tile([128, NBLK], BF16, name=f"acts{s}") for s in range(NC)]
              with ExitStack() as c2:
                # PSUM budget (16 KB/partition): pmm 2 banks (hid ph +
                # value pv, shared tag) | pl 2x2 banks (logits pairs) |
                # pso 2 banks (S@V) = 8 banks exactly
                pmm = c2.enter_context(tc.tile_pool(name="pmm", bufs=2, space="PSUM"))
                pl = c2.enter_context(tc.tile_pool(name="pl", bufs=2, space="PSUM"))
                pso = c2.enter_context(tc.tile_pool(name="pso", bufs=2, space="PSUM"))
                ep = c2.enter_context(tc.tile_pool(name="ep", bufs=16))
                rp = c2.enter_context(tc.tile_pool(name="rp", bufs=2))
                bp = c2.enter_context(tc.tile_pool(name="bp", bufs=2))
                ap_ = c2.enter_context(tc.tile_pool(name="ap", bufs=4))

                # ---- input DMAs (m-quarters so value proj starts early)
                nc.sync.dma_start(xc[:, 0:N_SEQ], xc_in[:, 0:N_SEQ])
                nc.sync.dma_start(xc[:, N_SEQ:NTOT], xc_in[:, N_SEQ:NTOT])
                QW = NTOT // 4
                for q in range(4):
                    for k in range(8):
                        nc.sync.dma_start(
                            xts[k][:, q * QW : (q + 1) * QW],
                            xt_in[k * 128 : (k + 1) * 128, q * QW : (q + 1) * QW],
                        )
                    if q == 1 and _rep == 0:
                        # wot only needed at P4; stream it behind the x load
                        nc.sync.dma_start(wot[:], wot_in[:])

                # ---- hid for all blocks (cheap, unblocks the exp stream)
                for nb in range(NB):
                    ph = pmm.tile([128, NBLK], F32, name="ph", tag="pm")
                    nc.tensor.matmul(
                        ph[:],
                        w1blk[:],
                        xc[:, nb * NBLK : (nb + 1) * NBLK],
                        start=True,
                        stop=True,
                        skip_group_check=True,
                    )
                    # relu with bias on DVE: out = max(ph + b1, 0)
                    nc.vector.tensor_scalar(
                        hidT[:, nb * NBLK : (nb + 1) * NBLK],
                        ph[:],
                        b1t[:, 0:1],
                        0.0,
                        ALU.add,
                        ALU.max,
                    )

                # ---- value proj chunk emitter (m on out partitions) ----
                def value_chunk(mc):
                    b, j = divmod(mc, MC)
                    pv = pmm.tile([128, NBLK], F32, name="pv", tag="pm")
                    for k in range(8):
                        nc.tensor.matmul(
                            pv[:, 0:128],
                            xts[k][:, mc * 128 : (mc + 1) * 128],
                            wv[:, k * 128 : (k + 1) * 128],
                            start=(k == 0),
                            stop=(k == 7),
                            skip_group_check=True,
                        )
                    # vh chunk = pv * exp(b2); den col untouched
                    for h in range(2):
                        nc.vector.tensor_scalar_mul(
                            vh_ap(b, j, h * 66, h * 66 + D),
                            pv[:, h * D : (h + 1) * D],
                            eb2[:, j : j + 1],
                        )

                # ---- P2 block pieces -----------------------------------
                def logits_block(h, nb):
                    eqs = []
                    for t in range(MC // 2):
                        eq = ep.tile([128, 2 * NBLK], BF16, name="eq", tag="eq")
                        plt = pl.tile([128, 2 * NBLK], F32, name="plt")
                        for u in range(2):
                            nc.tensor.matmul(
                                plt[:, u * NBLK : (u + 1) * NBLK],
                                w2t[
                                    h * D : (h + 1) * D,
                                    (2 * t + u) * 128 : (2 * t + u + 1) * 128,
                                ],
                                hidT[h * D : (h + 1) * D, nb * NBLK : (nb + 1) * NBLK],
                                start=True,
                                stop=True,
                                skip_group_check=True,
                            )
                        nc.scalar.activation(eq[:], plt[:], AF.Exp)
                        eqs.append(eq)
                    return eqs

                def sv_block(h, nb, eqs):
                    b = nb // (NB // B)
                    po = pso.tile([128, NBLK], F32, name="po")
                    for j in range(MC):
                        nc.tensor.matmul(
                            po[0:66, :],
                            vh_ap(b, j, h * 66, (h + 1) * 66),
                            eqs[j // 2][:, (j % 2) * NBLK : (j % 2 + 1) * NBLK],
                            start=(j == 0),
                            stop=(j == MC - 1),
                            skip_group_check=True,
                        )
                    # normalize: act = po[0:64] * bcast(1/den), den = po[64].
                    # broadcast via a K=1 matmul into the pmm pool (idle
                    # once the value projection drains)
                    rcp = rp.tile([66, NBLK], F32R, name="rcp")
                    with nc.allow_low_precision(
                        reason="softmax denominator reciprocal; f32r mantissa is plenty"
                    ):
                        nc.vector.reciprocal(rcp[64:66, :], po[64:66, :])
                    pb = pmm.tile([128, NBLK], F32, name="pb", tag="pm")
                    nc.tensor.matmul(
                        pb[:],
                        ones2[64:66, :],
                        rcp[64:66, :],
                        start=True,
                        stop=True,
                        skip_group_check=True,
                    )
                    pbs = bp.tile([D, NBLK], BF16, name="pbs")
                    nc.vector.tensor_copy(pbs[:], pb[0:D, :])
                    act = ap_.tile([D, NBLK], BF16, name="act")
                    nc.vector.tensor_mul(act[:], po[0:D, :], pbs[:])
                    nc.sync.dma_start(a2a_send[h][nb], act[:])

                # ---- emission schedule ---------------------------------
                # logits/exp of the first two blocks go first (ACT busy from
                # ~2us); value b0 streams in behind the xt DMAs; from there a
                # one-block logits->S@V skew keeps ACT fed, with batch-1
                # value chunks filling PE slack in the head-0 blocks.
                def fire_a2a(h):
                    if phases in ("1", "2"):
                        return
                    nc.gpsimd.collective_compute(
                        "AllToAll",
                        mybir.AluOpType.bypass,
                        ins=[a2a_send[h][:].opt()],
                        outs=[a2a_recv[h][:].opt()],
                        replica_groups=[list(range(NC))],
                    )
                    if phases not in ("3",):
                        for s in range(NC):
                            nc.sync.dma_start(
                                acts[s][h * D : (h + 1) * D, :], a2a_recv[h][s]
                            )

                order = [(0, nb) for nb in range(NB)] + [(1, nb) for nb in range(NB)]
                eqmap = {}
                eqmap[order[0]] = logits_block(*order[0])
                eqmap[order[1]] = logits_block(*order[1])
                for mc in range(MC):
                    value_chunk(mc)
                for i in range(len(order)):
                    if 2 <= i + 1 < len(order):
                        eqmap[order[i + 1]] = logits_block(*order[i + 1])
                    h, nb = order[i]
                    sv_block(h, nb, eqmap.pop((h, nb)))
                    if (h, nb) == (0, NB - 1):
                        fire_a2a(0)
                    elif (h, nb) == (1, NB - 1):
                        fire_a2a(1)
                    if h == 0 and nb < NB // 2:
                        for mc in range(MC + 4 * nb, MC + 4 * (nb + 1)):
                            value_chunk(mc)

              if phases in ("1", "2", "3"):
                continue

              # ---- P4: output projection ------------------------------
              with ExitStack() as c4:
                pw = c4.enter_context(tc.tile_pool(name="pw", bufs=2, space="PSUM"))
                obp = c4.enter_context(tc.tile_pool(name="obp", bufs=2))

                for t in range(NBLK // 128):
                    pt = pw.tile([128, FEAT], F32, name="pt")
                    for s in range(NC):
                        for u in range(2):
                            nc.tensor.matmul(
                                pt[:, u * NBLK : (u + 1) * NBLK],
                                acts[s][:, t * 128 : (t + 1) * 128],
                                wot[:, s * FEAT + u * NBLK : s * FEAT + (u + 1) * NBLK],
                                start=(s == 0),
                                stop=(s == NC - 1),
                                skip_group_check=True,
                            )
                    ob = obp.tile([128, FEAT], BF16, name="ob")
                    if t % 2 == 0:
                        nc.vector.tensor_copy(ob[:, 0:FEAT], pt[:])
                    else:
                        nc.scalar.copy(ob[:, 0:FEAT], pt[:])
                    nc.sync.dma_start(out_ext[t * 128 : (t + 1) * 128, :], ob[:])

    _split_sem_waits(nc)
    return nc


_CACHE = {}


def _get_program(reps=1, phases="A"):
    key = ("nc", reps, phases)
    if key not in _CACHE:
        _CACHE[key] = _build(reps, phases)
    return _CACHE[key]


def kernel(x, W1, b1, W2, b2, Wv, Wo, _run_kwargs=None):
    bf = ml_dtypes.bfloat16
    x = np.asarray(x, dtype=np.float32)
    W1 = np.asarray(W1, dtype=np.float32)
    b1 = np.asarray(b1, dtype=np.float32)
    W2 = np.asarray(W2, dtype=np.float32)
    b2 = np.asarray(b2, dtype=np.float32)
    Wv = np.asarray(Wv, dtype=np.float32)
    Wo = np.asarray(Wo, dtype=np.float32)

    xt = np.ascontiguousarray(x.reshape(NTOT, FEAT).T).astype(bf)  # [1024, 4096]
    w1blk = np.zeros((128, 128), np.float32)
    w1blk[0:D, 0:D] = W1.T
    w1blk[D:128, D:128] = W1.T
    w1blk = w1blk.astype(bf)
    b1t = np.concatenate([b1, b1]).reshape(128, 1).astype(np.float32)
    w2t = np.concatenate([W2.T, W2.T], axis=0).astype(bf)  # [128, 2048]
    eb2 = np.exp(b2.astype(np.float64)).astype(np.float32).reshape(MC, 128).T
    eb2 = np.ascontiguousarray(eb2)  # [128, 16]
    wot = (
        Wo.T.reshape(NC, 128, FEAT).transpose(1, 0, 2).reshape(128, NC * FEAT)
    ).astype(bf)

    in_maps = []
    for c in range(NC):
        wv_c = (
            Wv[c * 128 : (c + 1) * 128, :].T
            .reshape(8, 128, 128).transpose(1, 0, 2).reshape(128, FEAT)
        ).astype(bf)
        in_maps.append(
            {
                "xt": xt,
                "xc": np.ascontiguousarray(xt[c * 128 : (c + 1) * 128, :]),
                "wv": wv_c,
                "w1blk": w1blk,
                "b1": b1t,
                "w2t": w2t,
                "eb2": eb2,
                "wot": wot,
            }
        )

    import os
    nc = _get_program(
        int(os.environ.get("KERNEL_REPS", "1")), os.environ.get("KERNEL_PHASES", "A")
    )
    res = run_bass_kernel_spmd(
        nc, in_maps, list(range(NC)), **(_run_kwargs or {})
    )
    out = np.concatenate(
        [res.results[c]["out"].astype(np.float32) for c in range(NC)], axis=0
    )
    if _run_kwargs:
        kernel.last_results = res
    return out.reshape(B, N_SEQ, FEAT)
